# revision 1
# baseline (speedup 1.0000x reference)
import numpy as np
import jax
import jax.numpy as jnp

# nn_DWSABlock config (hardcoded; kernel.py must be self-contained)
B, C, H, W = 16, 256, 128, 128
RED, MID, NHEADS, G, WS = 64, 32, 4, 4, 8
SCALE = (MID // NHEADS) ** -0.5
EPS = 1e-5
NC = 8  # data-parallel over batch: 2 images per NeuronCore, weights replicated


def _forward_shard(x, W1, b1, Wq, bq, Wk, bk, Wv, bv, Wo, bo, W2, b2):
    # x: (B/NC, C, H, W). All small weights pre-folded host-side into dense mats.
    xr = jnp.einsum('oc,bchw->bohw', W1, x) + b1[None, :, None, None]
    nH, nW = H // WS, W // WS
    xw = xr.reshape(-1, RED, nH, WS, nW, WS)
    xw = xw.transpose(0, 2, 4, 1, 3, 5).reshape(-1, RED, WS * WS)

    q = jnp.einsum('or,brn->bon', Wq, xw) + bq[None, :, None]
    k = jnp.einsum('or,brn->bon', Wk, xw) + bk[None, :, None]
    v = jnp.einsum('or,brn->bon', Wv, xw) + bv[None, :, None]

    def split_heads(t):
        Bw = t.shape[0]
        return t.reshape(Bw, NHEADS, MID // NHEADS, WS * WS).transpose(0, 1, 3, 2)

    q, k, v = split_heads(q), split_heads(k), split_heads(v)
    attn = jax.nn.softmax(jnp.einsum('bhnd,bhmd->bhnm', q, k) * SCALE, axis=-1)
    out = jnp.einsum('bhnm,bhmd->bhnd', attn, v)
    Bw = out.shape[0]
    out = out.transpose(0, 1, 3, 2).reshape(Bw, MID, WS * WS)
    out = jnp.einsum('om,bmn->bon', Wo, out) + bo[None, :, None]

    out = out.reshape(-1, nH, nW, RED, WS, WS)
    out = out.transpose(0, 3, 1, 4, 2, 5).reshape(-1, RED, H, W)
    out = jnp.einsum('or,brhw->bohw', W2, out) + b2[None, :, None, None]
    return x + out


_pmapped = None


def _get_pmapped():
    global _pmapped
    if _pmapped is None:
        _pmapped = jax.pmap(_forward_shard, in_axes=(0,) + (None,) * 12)
    return _pmapped


def _fold(inputs):
    f = lambda k: np.asarray(inputs[k], np.float32)
    inv_in = f('bn_in_gamma') / np.sqrt(f('bn_in_var') + EPS)
    add_in = f('bn_in_beta') - f('bn_in_mean') * inv_in
    W1 = f('in_proj_w') * inv_in[None, :]            # (RED, C)
    b1 = f('in_proj_w') @ add_in                     # (RED,)

    # grouped 1x1 convs -> dense block-diagonal matrices
    qk = f('qk_base_w')                              # (G, MID/G, RED/G)
    Qb = np.zeros((MID, RED), np.float32)
    for g in range(G):
        Qb[g * (MID // G):(g + 1) * (MID // G),
           g * (RED // G):(g + 1) * (RED // G)] = qk[g]
    Wq = f('q_head_w') @ Qb                          # (MID, RED)
    bq = f('q_head_b')
    Wk = f('k_head_w') @ Qb
    bk = f('k_head_b')

    vw = f('v_w')
    Wv = np.zeros((MID, RED), np.float32)
    for g in range(G):
        Wv[g * (MID // G):(g + 1) * (MID // G),
           g * (RED // G):(g + 1) * (RED // G)] = vw[g]
    bv = f('v_b')

    ow = f('o_w')                                    # (G, RED/G, MID/G)
    Wo = np.zeros((RED, MID), np.float32)
    for g in range(G):
        Wo[g * (RED // G):(g + 1) * (RED // G),
           g * (MID // G):(g + 1) * (MID // G)] = ow[g]
    bo = f('o_b')

    inv_out = f('bn_out_gamma') / np.sqrt(f('bn_out_var') + EPS)
    add_out = f('bn_out_beta') - f('bn_out_mean') * inv_out
    s = 1.0 / (1.0 + np.exp(-np.float32(inputs['alpha'])))
    W2 = (s * inv_out[:, None] * f('out_proj_w')).astype(np.float32)  # (C, RED)
    b2 = (s * add_out).astype(np.float32)
    return W1, b1, Wq, bq, Wk, bk, Wv, bv, Wo, bo, W2, b2


def kernel(**inputs) -> np.ndarray:
    x = np.asarray(inputs['x'], np.float32)
    weights = _fold(inputs)
    xs = x.reshape(NC, B // NC, C, H, W)             # shard batch across 8 cores
    out = _get_pmapped()(xs, *weights)
    return np.asarray(out).reshape(B, C, H, W).astype(np.float32)


# revision 2
# speedup vs baseline: 367.3104x; 367.3104x over previous
import numpy as np
import jax
import jax.numpy as jnp

# nn_DWSABlock config (hardcoded; kernel.py must be self-contained)
B, C, H, W = 16, 256, 128, 128
RED, MID, NHEADS, G, WS = 64, 32, 4, 4, 8
HD = MID // NHEADS
SCALE = HD ** -0.5
EPS = 1e-5
NC = 8  # data-parallel over batch: 2 images per NeuronCore, weights replicated
N = WS * WS
nH, nW = H // WS, W // WS


def _forward_shard(x, W1, b1, M4, U4, Wv, bv, Wo, bo, W2, b2):
    # x: (Bs, C, H, W). Attention scores use host-folded per-head matrices:
    # S_h = xw^T (M_h xw) + (U_h^T xw broadcast over n); terms constant along
    # the softmax axis cancel, so q/k projections never materialize.
    Bs = x.shape[0]
    xf = x.reshape(Bs, C, H * W)
    xr = jnp.einsum('oc,bcn->bon', W1, xf) + b1[None, :, None]

    xw = xr.reshape(Bs, RED, nH, WS, nW, WS)
    xw = xw.transpose(0, 2, 4, 1, 3, 5).reshape(Bs * nH * nW, RED, N)

    T = jnp.einsum('hsr,brm->bhsm', M4, xw)           # (Bw, h, RED, N)
    S = jnp.einsum('bsn,bhsm->bhnm', xw, T)           # (Bw, h, N, N)
    S = S + jnp.einsum('hr,brm->bhm', U4, xw)[:, :, None, :]
    A = jax.nn.softmax(S, axis=-1)

    v = jnp.einsum('or,brn->bon', Wv, xw) + bv[None, :, None]
    v = v.reshape(-1, NHEADS, HD, N).transpose(0, 1, 3, 2)  # (Bw, h, N, HD)
    out = jnp.einsum('bhnm,bhmd->bhnd', A, v)
    out = out.transpose(0, 1, 3, 2).reshape(-1, MID, N)
    out = jnp.einsum('om,bmn->bon', Wo, out) + bo[None, :, None]

    out = out.reshape(Bs, nH, nW, RED, WS, WS)
    out = out.transpose(0, 3, 1, 4, 2, 5).reshape(Bs, RED, H * W)
    out = jnp.einsum('or,brn->bon', W2, out) + b2[None, :, None]
    return x + out.reshape(Bs, C, H, W)


_pmapped = None


def _get_pmapped():
    global _pmapped
    if _pmapped is None:
        _pmapped = jax.pmap(_forward_shard, in_axes=(0,) + (None,) * 10)
    return _pmapped


def _fold(inputs):
    f = lambda k: np.asarray(inputs[k], np.float32)
    inv_in = f('bn_in_gamma') / np.sqrt(f('bn_in_var') + EPS)
    add_in = f('bn_in_beta') - f('bn_in_mean') * inv_in
    W1 = f('in_proj_w') * inv_in[None, :]            # (RED, C)
    b1 = f('in_proj_w') @ add_in                     # (RED,)

    # grouped 1x1 convs -> dense block-diagonal matrices
    qk = f('qk_base_w')                              # (G, MID/G, RED/G)
    Qb = np.zeros((MID, RED), np.float32)
    for g in range(G):
        Qb[g * (MID // G):(g + 1) * (MID // G),
           g * (RED // G):(g + 1) * (RED // G)] = qk[g]
    Wq = f('q_head_w') @ Qb                          # (MID, RED)
    bq = f('q_head_b')
    Wk = f('k_head_w') @ Qb
    bk = f('k_head_b')

    # per-head folded score matrices: S_h = (Wq_h x + bq_h)^T (Wk_h x + bk_h)
    # -> x^T (SCALE Wq_h^T Wk_h) x  +  (SCALE bq_h^T Wk_h) x  (+ n-only terms
    # and constants, which cancel in softmax over m)
    M4 = np.zeros((NHEADS, RED, RED), np.float32)
    U4 = np.zeros((NHEADS, RED), np.float32)
    for h in range(NHEADS):
        Wqh = Wq[h * HD:(h + 1) * HD]                # (HD, RED)
        Wkh = Wk[h * HD:(h + 1) * HD]
        M4[h] = SCALE * (Wqh.T @ Wkh)
        U4[h] = SCALE * (bq[h * HD:(h + 1) * HD] @ Wkh)

    vw = f('v_w')
    Wv = np.zeros((MID, RED), np.float32)
    for g in range(G):
        Wv[g * (MID // G):(g + 1) * (MID // G),
           g * (RED // G):(g + 1) * (RED // G)] = vw[g]
    bv = f('v_b')

    ow = f('o_w')                                    # (G, RED/G, MID/G)
    Wo = np.zeros((RED, MID), np.float32)
    for g in range(G):
        Wo[g * (RED // G):(g + 1) * (RED // G),
           g * (MID // G):(g + 1) * (MID // G)] = ow[g]
    bo = f('o_b')

    inv_out = f('bn_out_gamma') / np.sqrt(f('bn_out_var') + EPS)
    add_out = f('bn_out_beta') - f('bn_out_mean') * inv_out
    s = 1.0 / (1.0 + np.exp(-np.float32(inputs['alpha'])))
    W2 = (s * inv_out[:, None] * f('out_proj_w')).astype(np.float32)  # (C, RED)
    b2 = (s * add_out).astype(np.float32)
    return W1, b1, M4, U4, Wv, bv, Wo, bo, W2, b2


def kernel(**inputs) -> np.ndarray:
    x = np.asarray(inputs['x'], np.float32)
    weights = _fold(inputs)
    xs = x.reshape(NC, B // NC, C, H, W)             # shard batch across 8 cores
    out = _get_pmapped()(xs, *weights)
    return np.asarray(out).reshape(B, C, H, W).astype(np.float32)
